# revision 3
# baseline (speedup 1.0000x reference)
"""AttentionBlock (GroupNorm -> 1x1 qkv -> spatial softmax attention -> 1x1 proj
-> residual) on 8 TRN2 NeuronCores, pure data parallel over batch B=8.

Per-core shapes: x [C=128, HW=4096]. All heavy compute in bf16 on TensorE with
f32 PSUM accumulation; exp on ScalarE; GroupNorm folded into the qkv weights.
"""

import numpy as np

import concourse.bass as bass
import concourse.bacc as bacc
import concourse.tile as tile
from concourse import mybir
from concourse.bass_utils import run_bass_kernel_spmd

F32 = mybir.dt.float32
BF16 = mybir.dt.bfloat16
AF = mybir.ActivationFunctionType

B = 8
C = 128
H = 64
W = 64
HW = H * W          # 4096
NG = 32             # groups
GS = C // NG        # 4 channels per group
EPS = 1e-5
SCALE = float(C) ** -0.5

CW = 1024           # attention chunk width (exp batch; 2 PSUM banks)
PW = 512            # matmul piece width (1 PSUM bank)
NCHUNK = HW // CW   # 4
MT = HW // C        # 32 m-tiles (key tiles)


def _body(tc, x_d, gamma_d, beta_d, wqkvT_d, bqkv3_d, woutT_d, bout_d, G_d, GT_d, out_d):
    nc = tc.nc
    with (
        tc.tile_pool(name="singles", bufs=1) as singles,
        tc.tile_pool(name="smallw", bufs=2) as smallw,
        tc.tile_pool(name="dram", bufs=2, space="DRAM") as drampool,
    ):
        # ---- persistent SBUF tiles ----
        x_sb = singles.tile([C, HW], F32)      # raw x, later x + bfinal
        xb16 = singles.tile([C, HW], BF16)
        q_sb = singles.tile([C, HW], BF16)
        k_sb = singles.tile([C, HW], BF16)
        vt_sb = singles.tile([C, HW], BF16)    # 32 blocks of [128m, 128c]
        wq_sb = singles.tile([C, 3 * C], F32)  # w_qkv^T
        wadj = singles.tile([C, 3 * C], BF16)  # groupnorm-folded w_qkv^T
        wo_sb = singles.tile([C, C], F32)      # w_out^T
        wo16 = singles.tile([C, C], BF16)
        gamma_sb = singles.tile([C, 1], F32)
        beta_sb = singles.tile([C, 1], F32)
        bqkv3_sb = singles.tile([C, 3], F32)
        bout_sb = singles.tile([C, 1], F32)
        G_sb = singles.tile([C, NG], F32)
        GT_sb = singles.tile([NG, C], F32)
        ones1 = singles.tile([C, 1], BF16)     # lhsT for row-sum matmuls
        a_sb = smallw.tile([C, 1], F32, tag="aff")
        aq_sb = smallw.tile([C, 1], F32, tag="aff")
        bvec = smallw.tile([C, 1], F32, tag="aff")
        bq_sb = smallw.tile([C, 1], F32, tag="aff")
        bv_sb = smallw.tile([C, 1], F32, tag="aff")
        bfinal = smallw.tile([C, 1], F32, tag="aff")

        # ---- input DMAs ----
        for i in range(4):
            nc.sync.dma_start(x_sb[:, i * CW:(i + 1) * CW], x_d[:, i * CW:(i + 1) * CW])
        nc.sync.dma_start(wq_sb, wqkvT_d)
        nc.sync.dma_start(wo_sb, woutT_d)
        nc.sync.dma_start(gamma_sb, gamma_d)
        nc.sync.dma_start(beta_sb, beta_d)
        nc.sync.dma_start(bqkv3_sb, bqkv3_d)
        nc.sync.dma_start(bout_sb, bout_d)
        nc.sync.dma_start(G_sb, G_d)
        nc.sync.dma_start(GT_sb, GT_d)
        nc.vector.memset(ones1, 1.0)
        nc.vector.tensor_copy(wo16, wo_sb)

        for i in range(4):
            nc.vector.tensor_copy(xb16[:, i * CW:(i + 1) * CW], x_sb[:, i * CW:(i + 1) * CW])

        # ---- groupnorm stats, folded into qkv weights ----
        with tc.tile_pool(name="statsp", bufs=2, space="PSUM") as statsp:
            stats = smallw.tile([C, 8, 6], F32, tag="stats")
            for i in range(8):
                nc.vector.bn_stats(stats[:, i, :], x_sb[:, i * PW:(i + 1) * PW])
            mv = smallw.tile([C, 2], F32, tag="mv")
            nc.vector.bn_aggr(mv, stats)
            # mom = [E[x], E[x^2]] per channel
            mom = smallw.tile([C, 2], F32, tag="mom")
            nc.vector.tensor_copy(mom[:, 0:1], mv[:, 0:1])
            tmp = smallw.tile([C, 1], F32, tag="tmp1")
            nc.vector.tensor_mul(tmp, mv[:, 0:1], mv[:, 0:1])
            nc.vector.tensor_add(mom[:, 1:2], mv[:, 1:2], tmp)
            # group averages via indicator matmul (G entries = 1/GS)
            gs_ps = statsp.tile([NG, 2], F32, tag="st")
            nc.tensor.matmul(gs_ps, G_sb, mom)
            gs = smallw.tile([NG, 2], F32, tag="gs")
            nc.vector.tensor_copy(gs, gs_ps)
            gsq = smallw.tile([NG, 1], F32, tag="gsq")
            nc.vector.tensor_mul(gsq, gs[:, 0:1], gs[:, 0:1])
            gvar = smallw.tile([NG, 1], F32, tag="gvar")
            nc.vector.tensor_sub(gvar, gs[:, 1:2], gsq)
            eps_sb = smallw.tile([NG, 1], F32, tag="eps")
            nc.vector.memset(eps_sb, EPS)
            gstd = smallw.tile([NG, 1], F32, tag="gstd")
            nc.scalar.activation(gstd, gvar, AF.Sqrt, bias=eps_sb)
            grstd = smallw.tile([NG, 1], F32, tag="grstd")
            nc.vector.reciprocal(grstd, gstd)
            pair = smallw.tile([NG, 2], F32, tag="pair")
            nc.vector.tensor_copy(pair[:, 0:1], grstd)
            nmean = smallw.tile([NG, 1], F32, tag="nmean")
            nc.vector.tensor_mul(nmean, gs[:, 0:1], grstd)
            nc.vector.tensor_scalar_mul(pair[:, 1:2], nmean, -1.0)
            # broadcast group (rstd, -mean*rstd) back to channels
            cp_ps = statsp.tile([C, 2], F32, tag="st")
            nc.tensor.matmul(cp_ps, GT_sb, pair)
            cp = smallw.tile([C, 2], F32, tag="cp")
            nc.vector.tensor_copy(cp, cp_ps)
            # xn = a*x + b per channel; fold into weights
            nc.vector.tensor_mul(a_sb, gamma_sb, cp[:, 0:1])
            nc.vector.tensor_scalar_mul(aq_sb, a_sb, SCALE)
            nc.vector.tensor_mul(bvec, gamma_sb, cp[:, 1:2])
            nc.vector.tensor_add(bvec, bvec, beta_sb)
            nc.vector.tensor_scalar_mul(wadj[:, 0:C], wq_sb[:, 0:C], aq_sb)
            nc.vector.tensor_scalar_mul(wadj[:, C:3 * C], wq_sb[:, C:3 * C], a_sb)
            # bq' = SCALE*(W_q @ bvec + b_q); k bias drops (softmax shift invariance)
            b1 = statsp.tile([C, 1], F32, tag="st")
            nc.tensor.matmul(b1, wq_sb[:, 0:C], bvec)
            nc.vector.tensor_add(bq_sb, b1, bqkv3_sb[:, 0:1])
            nc.vector.tensor_scalar_mul(bq_sb, bq_sb, SCALE)
            # v bias: bv' = W_v @ bvec + b_v; folded into final bias
            b2 = statsp.tile([C, 1], F32, tag="st")
            nc.tensor.matmul(b2, wq_sb[:, 2 * C:3 * C], bvec)
            nc.vector.tensor_add(bv_sb, b2, bqkv3_sb[:, 2:3])
            b3 = statsp.tile([C, 1], F32, tag="st")
            nc.tensor.matmul(b3, wo_sb, bv_sb)
            nc.vector.tensor_add(bfinal, b3, bout_sb)

        # x_sb becomes (x + bfinal): the residual-plus-constant term
        nc.vector.tensor_scalar_add(x_sb, x_sb, bfinal)

        with (
            tc.tile_pool(name="spsum", bufs=2, space="PSUM") as s_pool,
            tc.tile_pool(name="opsum", bufs=1, space="PSUM") as o_pool,
            tc.tile_pool(name="r0psum", bufs=1, space="PSUM") as r0_pool,
            tc.tile_pool(name="r1psum", bufs=1, space="PSUM") as r1_pool,
            tc.tile_pool(name="ptp", bufs=3) as pt_pool,
            tc.tile_pool(name="osb", bufs=2) as osb_pool,
            tc.tile_pool(name="rbcp", bufs=2) as rbc_pool,
            tc.tile_pool(name="finp", bufs=3) as fin_pool,
        ):
            # ---- q, k (channel-major) ----
            for i in range(4):
                ps = s_pool.tile([C, CW], F32, tag="s")
                nc.tensor.matmul(ps[:, 0:PW], wadj[:, 0:C], xb16[:, i * CW:i * CW + PW])
                nc.tensor.matmul(ps[:, PW:CW], wadj[:, 0:C], xb16[:, i * CW + PW:(i + 1) * CW])
                nc.scalar.activation(q_sb[:, i * CW:(i + 1) * CW], ps, AF.Identity, bias=bq_sb)
            for i in range(4):
                ps = s_pool.tile([C, CW], F32, tag="s")
                nc.tensor.matmul(ps[:, 0:PW], wadj[:, C:2 * C], xb16[:, i * CW:i * CW + PW])
                nc.tensor.matmul(ps[:, PW:CW], wadj[:, C:2 * C], xb16[:, i * CW + PW:(i + 1) * CW])
                nc.vector.tensor_copy(k_sb[:, i * CW:(i + 1) * CW], ps)
            # ---- V^T directly: block t is [m_local=128, c=128] ----
            for t in range(8):
                ps = s_pool.tile([C, PW], F32, tag="s")
                for j in range(4):
                    mt = 4 * t + j
                    nc.tensor.matmul(ps[:, j * C:(j + 1) * C],
                                     xb16[:, mt * C:(mt + 1) * C], wadj[:, 2 * C:3 * C])
                nc.vector.tensor_copy(vt_sb[:, t * PW:(t + 1) * PW], ps)

            # ---- attention ----
            for c in range(NCHUNK):
                n0 = c * CW
                o_ps = o_pool.tile([C, CW], F32)
                r0 = r0_pool.tile([1, PW], F32)
                r1 = r1_pool.tile([1, PW], F32)
                for m in range(MT):
                    sp = s_pool.tile([C, CW], F32, tag="s")
                    nc.tensor.matmul(sp[:, 0:PW], k_sb[:, m * C:(m + 1) * C], q_sb[:, n0:n0 + PW])
                    nc.tensor.matmul(sp[:, PW:CW], k_sb[:, m * C:(m + 1) * C], q_sb[:, n0 + PW:n0 + CW])
                    pt = pt_pool.tile([C, CW], BF16)
                    nc.scalar.activation(pt, sp, AF.Exp)
                    st = m == 0
                    en = m == MT - 1
                    nc.tensor.matmul(o_ps[:, 0:PW], vt_sb[:, m * C:(m + 1) * C], pt[:, 0:PW],
                                     start=st, stop=en)
                    nc.tensor.matmul(o_ps[:, PW:CW], vt_sb[:, m * C:(m + 1) * C], pt[:, PW:CW],
                                     start=st, stop=en)
                    nc.tensor.matmul(r0, ones1, pt[:, 0:PW], start=st, stop=en)
                    nc.tensor.matmul(r1, ones1, pt[:, PW:CW], start=st, stop=en)
                # softmax denominators -> 1/r, broadcast to 128 partitions via DRAM
                rinv = rbc_pool.tile([1, CW], F32, tag="rinv")
                nc.vector.reciprocal(rinv[:, 0:PW], r0)
                nc.vector.reciprocal(rinv[:, PW:CW], r1)
                rscratch = drampool.tile([1, CW], F32)
                nc.sync.dma_start(rscratch, rinv)
                rbc = rbc_pool.tile([C, CW], F32, tag="rbc")
                nc.sync.dma_start(rbc, rscratch[0].partition_broadcast(C))
                o_sb = osb_pool.tile([C, CW], BF16)
                nc.vector.tensor_copy(o_sb, o_ps)
                for h in range(2):
                    y_ps = s_pool.tile([C, PW], F32, tag="s")
                    nc.tensor.matmul(y_ps, wo16, o_sb[:, h * PW:(h + 1) * PW])
                    fin = fin_pool.tile([C, PW], F32)
                    nc.vector.tensor_mul(fin, y_ps, rbc[:, h * PW:(h + 1) * PW])
                    nc.vector.tensor_add(fin, fin, x_sb[:, n0 + h * PW:n0 + (h + 1) * PW])
                    nc.sync.dma_start(out_d[:, n0 + h * PW:n0 + (h + 1) * PW], fin)


def build():
    nc = bacc.Bacc("TRN2", target_bir_lowering=False, debug=False, num_devices=B)
    x_d = nc.dram_tensor("x", [C, HW], F32, kind="ExternalInput").ap()
    gamma_d = nc.dram_tensor("gamma", [C, 1], F32, kind="ExternalInput").ap()
    beta_d = nc.dram_tensor("beta", [C, 1], F32, kind="ExternalInput").ap()
    wqkvT_d = nc.dram_tensor("wqkvT", [C, 3 * C], F32, kind="ExternalInput").ap()
    bqkv3_d = nc.dram_tensor("bqkv3", [C, 3], F32, kind="ExternalInput").ap()
    woutT_d = nc.dram_tensor("woutT", [C, C], F32, kind="ExternalInput").ap()
    bout_d = nc.dram_tensor("bout", [C, 1], F32, kind="ExternalInput").ap()
    out_d = nc.dram_tensor("out", [C, HW], F32, kind="ExternalOutput").ap()

    G = np.zeros((C, NG), np.float32)
    G[np.arange(C), np.arange(C) // GS] = 1.0 / GS
    GT = np.zeros((NG, C), np.float32)
    GT[np.arange(C) // GS, np.arange(C)] = 1.0
    G_d = nc.inline_tensor(G, "Gmat").ap()
    GT_d = nc.inline_tensor(GT, "GTmat").ap()

    with tile.TileContext(nc) as tc:
        _body(tc, x_d, gamma_d, beta_d, wqkvT_d, bqkv3_d, woutT_d, bout_d, G_d, GT_d, out_d)
    nc.compile()
    return nc


_NC = None


def _get_nc():
    global _NC
    if _NC is None:
        _NC = build()
    return _NC


def _in_maps(x, gamma, beta, w_qkv, b_qkv, w_out, b_out):
    x = np.ascontiguousarray(np.asarray(x, np.float32)).reshape(B, C, HW)
    shared = {
        "gamma": np.ascontiguousarray(np.asarray(gamma, np.float32).reshape(C, 1)),
        "beta": np.ascontiguousarray(np.asarray(beta, np.float32).reshape(C, 1)),
        "wqkvT": np.ascontiguousarray(np.asarray(w_qkv, np.float32).T),
        "bqkv3": np.ascontiguousarray(np.asarray(b_qkv, np.float32).reshape(3, C).T),
        "woutT": np.ascontiguousarray(np.asarray(w_out, np.float32).T),
        "bout": np.ascontiguousarray(np.asarray(b_out, np.float32).reshape(C, 1)),
    }
    return [{"x": np.ascontiguousarray(x[b]), **shared} for b in range(B)]


def run(inputs, trace=False, **trace_kwargs):
    nc = _get_nc()
    maps = _in_maps(**inputs)
    res = run_bass_kernel_spmd(nc, maps, list(range(B)), trace=trace, **trace_kwargs)
    out = np.stack([np.asarray(res.results[i]["out"]) for i in range(B)])
    return out.reshape(B, C, H, W).astype(np.float32), res


def kernel(**inputs):
    out, _ = run(inputs)
    return out


# revision 12
# speedup vs baseline: 1.4226x; 1.4226x over previous
"""AttentionBlock (GroupNorm -> 1x1 qkv -> spatial softmax attention -> 1x1 proj
-> residual) on 8 TRN2 NeuronCores, pure data parallel over batch B=8.

Per-core shapes: x [C=128, HW=4096]. All heavy compute in bf16 on TensorE with
f32 PSUM accumulation; exp on ScalarE; GroupNorm folded into the qkv weights.
"""

import numpy as np

import concourse.bass as bass
import concourse.bacc as bacc
import concourse.tile as tile
from concourse import mybir
from concourse.bass_utils import run_bass_kernel_spmd

F32 = mybir.dt.float32
BF16 = mybir.dt.bfloat16
AF = mybir.ActivationFunctionType

B = 8
C = 128
H = 64
W = 64
HW = H * W          # 4096
NG = 32             # groups
GS = C // NG        # 4 channels per group
EPS = 1e-5
SCALE = float(C) ** -0.5

CW = 1024           # attention chunk width (exp batch; 2 PSUM banks)
PW = 512            # matmul piece width (1 PSUM bank)
NCHUNK = HW // CW   # 4
MT = HW // C        # 32 m-tiles (key tiles)


def _body(tc, x_d, gamma_d, beta_d, wqkvT_d, bqkv3_d, woutT_d, bout_d, G_d, GT_d, out_d):
    nc = tc.nc
    with (
        tc.tile_pool(name="singles", bufs=1) as singles,
        tc.tile_pool(name="smallw", bufs=2) as smallw,
        tc.tile_pool(name="dram", bufs=2, space="DRAM") as drampool,
    ):
        # ---- persistent SBUF tiles ----
        x_sb = singles.tile([C, HW], F32)      # raw x, later x + bfinal
        xb16 = singles.tile([C, HW], BF16)
        q_sb = singles.tile([C, HW], BF16)
        k_sb = singles.tile([C, HW], BF16)
        vt_sb = singles.tile([C, HW], BF16)    # 32 blocks of [128m, 128c]
        wq_sb = singles.tile([C, 3 * C], F32)  # w_qkv^T
        wadj = singles.tile([C, 3 * C], BF16)  # groupnorm-folded w_qkv^T
        wo_sb = singles.tile([C, C], F32)      # w_out^T
        wo16 = singles.tile([C, C], BF16)
        gamma_sb = singles.tile([C, 1], F32)
        beta_sb = singles.tile([C, 1], F32)
        bqkv3_sb = singles.tile([C, 3], F32)
        bout_sb = singles.tile([C, 1], F32)
        G_sb = singles.tile([C, NG], F32)
        GT_sb = singles.tile([NG, C], F32)
        ones1 = singles.tile([C, 1], BF16)     # lhsT for row-sum matmuls
        a_sb = smallw.tile([C, 1], F32, tag="aff")
        aq_sb = smallw.tile([C, 1], F32, tag="aff")
        bvec = smallw.tile([C, 1], F32, tag="aff")
        bq_sb = smallw.tile([C, 1], F32, tag="aff")
        bv_sb = smallw.tile([C, 1], F32, tag="aff")
        bfinal = smallw.tile([C, 1], F32, tag="aff")

        # ---- input DMAs ----
        for i in range(4):
            nc.sync.dma_start(x_sb[:, i * CW:(i + 1) * CW], x_d[:, i * CW:(i + 1) * CW])
        nc.sync.dma_start(wq_sb, wqkvT_d)
        nc.sync.dma_start(wo_sb, woutT_d)
        nc.sync.dma_start(gamma_sb, gamma_d)
        nc.sync.dma_start(beta_sb, beta_d)
        nc.sync.dma_start(bqkv3_sb, bqkv3_d)
        nc.sync.dma_start(bout_sb, bout_d)
        nc.sync.dma_start(G_sb, G_d)
        nc.sync.dma_start(GT_sb, GT_d)
        nc.vector.memset(ones1, 1.0)
        nc.vector.tensor_copy(wo16, wo_sb)

        for i in range(4):
            nc.vector.tensor_copy(xb16[:, i * CW:(i + 1) * CW], x_sb[:, i * CW:(i + 1) * CW])

        # ---- groupnorm stats, folded into qkv weights ----
        with tc.tile_pool(name="statsp", bufs=2, space="PSUM") as statsp:
            stats = smallw.tile([C, 8, 6], F32, tag="stats")
            for i in range(8):
                nc.vector.bn_stats(stats[:, i, :], x_sb[:, i * PW:(i + 1) * PW])
            mv = smallw.tile([C, 2], F32, tag="mv")
            nc.vector.bn_aggr(mv, stats)
            # mom = [E[x], E[x^2]] per channel
            mom = smallw.tile([C, 2], F32, tag="mom")
            nc.vector.tensor_copy(mom[:, 0:1], mv[:, 0:1])
            tmp = smallw.tile([C, 1], F32, tag="tmp1")
            nc.vector.tensor_mul(tmp, mv[:, 0:1], mv[:, 0:1])
            nc.vector.tensor_add(mom[:, 1:2], mv[:, 1:2], tmp)
            # group averages via indicator matmul (G entries = 1/GS)
            gs_ps = statsp.tile([NG, 2], F32, tag="st")
            nc.tensor.matmul(gs_ps, G_sb, mom)
            gs = smallw.tile([NG, 2], F32, tag="gs")
            nc.vector.tensor_copy(gs, gs_ps)
            gsq = smallw.tile([NG, 1], F32, tag="gsq")
            nc.vector.tensor_mul(gsq, gs[:, 0:1], gs[:, 0:1])
            gvar = smallw.tile([NG, 1], F32, tag="gvar")
            nc.vector.tensor_sub(gvar, gs[:, 1:2], gsq)
            eps_sb = smallw.tile([NG, 1], F32, tag="eps")
            nc.vector.memset(eps_sb, EPS)
            # rstd = exp(-0.5*ln(var+eps)): Log+Exp share one ACT table set
            glog = smallw.tile([NG, 1], F32, tag="glog")
            nc.scalar.activation(glog, gvar, AF.Ln, bias=eps_sb)
            grstd = smallw.tile([NG, 1], F32, tag="grstd")
            nc.scalar.activation(grstd, glog, AF.Exp, scale=-0.5)
            pair = smallw.tile([NG, 2], F32, tag="pair")
            nc.vector.tensor_copy(pair[:, 0:1], grstd)
            nmean = smallw.tile([NG, 1], F32, tag="nmean")
            nc.vector.tensor_mul(nmean, gs[:, 0:1], grstd)
            nc.vector.tensor_scalar_mul(pair[:, 1:2], nmean, -1.0)
            # broadcast group (rstd, -mean*rstd) back to channels
            cp_ps = statsp.tile([C, 2], F32, tag="st")
            nc.tensor.matmul(cp_ps, GT_sb, pair)
            cp = smallw.tile([C, 2], F32, tag="cp")
            nc.vector.tensor_copy(cp, cp_ps)
            # xn = a*x + b per channel; fold into weights
            nc.vector.tensor_mul(a_sb, gamma_sb, cp[:, 0:1])
            nc.vector.tensor_scalar_mul(aq_sb, a_sb, SCALE)
            nc.vector.tensor_mul(bvec, gamma_sb, cp[:, 1:2])
            nc.vector.tensor_add(bvec, bvec, beta_sb)
            nc.vector.tensor_scalar_mul(wadj[:, 0:C], wq_sb[:, 0:C], aq_sb)
            nc.vector.tensor_scalar_mul(wadj[:, C:3 * C], wq_sb[:, C:3 * C], a_sb)
            # bq' = SCALE*(W_q @ bvec + b_q); k bias drops (softmax shift invariance)
            b1 = statsp.tile([C, 1], F32, tag="st")
            nc.tensor.matmul(b1, wq_sb[:, 0:C], bvec)
            nc.vector.tensor_add(bq_sb, b1, bqkv3_sb[:, 0:1])
            nc.vector.tensor_scalar_mul(bq_sb, bq_sb, SCALE)
            # v bias: bv' = W_v @ bvec + b_v; folded into final bias
            b2 = statsp.tile([C, 1], F32, tag="st")
            nc.tensor.matmul(b2, wq_sb[:, 2 * C:3 * C], bvec)
            nc.vector.tensor_add(bv_sb, b2, bqkv3_sb[:, 2:3])
            b3 = statsp.tile([C, 1], F32, tag="st")
            nc.tensor.matmul(b3, wo_sb, bv_sb)
            nc.vector.tensor_add(bfinal, b3, bout_sb)

        # x_sb becomes (x + bfinal): the residual-plus-constant term
        nc.vector.tensor_scalar_add(x_sb, x_sb, bfinal)

        with (
            tc.tile_pool(name="spsum", bufs=2, space="PSUM") as s_pool,
            tc.tile_pool(name="opsum", bufs=1, space="PSUM") as o_pool,
            tc.tile_pool(name="r0psum", bufs=1, space="PSUM") as r0_pool,
            tc.tile_pool(name="r1psum", bufs=1, space="PSUM") as r1_pool,
            tc.tile_pool(name="ptp", bufs=4) as pt_pool,
            tc.tile_pool(name="osb", bufs=2) as osb_pool,
            tc.tile_pool(name="rbcp", bufs=2) as rbc_pool,
            tc.tile_pool(name="finp", bufs=3) as fin_pool,
        ):
            # ---- q, k (channel-major) ----
            for i in range(4):
                ps = s_pool.tile([C, CW], F32, tag="s")
                nc.tensor.matmul(ps[:, 0:PW], wadj[:, 0:C], xb16[:, i * CW:i * CW + PW])
                nc.tensor.matmul(ps[:, PW:CW], wadj[:, 0:C], xb16[:, i * CW + PW:(i + 1) * CW])
                nc.scalar.activation(q_sb[:, i * CW:(i + 1) * CW], ps, AF.Identity, bias=bq_sb)
            for i in range(4):
                ps = s_pool.tile([C, CW], F32, tag="s")
                nc.tensor.matmul(ps[:, 0:PW], wadj[:, C:2 * C], xb16[:, i * CW:i * CW + PW])
                nc.tensor.matmul(ps[:, PW:CW], wadj[:, C:2 * C], xb16[:, i * CW + PW:(i + 1) * CW])
                nc.vector.tensor_copy(k_sb[:, i * CW:(i + 1) * CW], ps)
            # ---- V^T directly: block t is [m_local=128, c=128] ----
            for t in range(8):
                ps = s_pool.tile([C, PW], F32, tag="s")
                for j in range(4):
                    mt = 4 * t + j
                    nc.tensor.matmul(ps[:, j * C:(j + 1) * C],
                                     xb16[:, mt * C:(mt + 1) * C], wadj[:, 2 * C:3 * C])
                nc.vector.tensor_copy(vt_sb[:, t * PW:(t + 1) * PW], ps)

            # ---- attention ----
            def emit_s(c, m, sp_live):
                n0 = c * CW
                sp = s_pool.tile([C, CW], F32, tag="s")
                nc.tensor.matmul(sp[:, 0:PW], k_sb[:, m * C:(m + 1) * C], q_sb[:, n0:n0 + PW])
                nc.tensor.matmul(sp[:, PW:CW], k_sb[:, m * C:(m + 1) * C], q_sb[:, n0 + PW:n0 + CW])
                sp_live[m] = sp

            def emit_tail(c, o_ps, r0, r1):
                # softmax denominators -> 1/r, broadcast to 128 partitions via
                # a DRAM bounce; then projection, normalize, residual, DMA out.
                n0 = c * CW
                rinv = rbc_pool.tile([1, CW], F32, tag="rinv")
                nc.vector.reciprocal_approx_fast(rinv[:, 0:PW], r0)
                nc.vector.reciprocal_approx_fast(rinv[:, PW:CW], r1)
                rscratch = drampool.tile([1, CW], F32)
                nc.sync.dma_start(rscratch, rinv)
                rbc = rbc_pool.tile([C, CW], F32, tag="rbc")
                nc.sync.dma_start(rbc, rscratch[0].partition_broadcast(C))
                o_sb = osb_pool.tile([C, CW], BF16)
                nc.vector.tensor_copy(o_sb, o_ps)
                for h in range(2):
                    y_ps = s_pool.tile([C, PW], F32, tag="s")
                    nc.tensor.matmul(y_ps, wo16, o_sb[:, h * PW:(h + 1) * PW])
                    fin = fin_pool.tile([C, PW], F32)
                    nc.vector.tensor_mul(fin, y_ps, rbc[:, h * PW:(h + 1) * PW])
                    nc.vector.tensor_add(fin, fin, x_sb[:, n0 + h * PW:n0 + (h + 1) * PW])
                    nc.sync.dma_start(out_d[:, n0 + h * PW:n0 + (h + 1) * PW], fin)

            pending = None
            for c in range(NCHUNK):
                o_ps = o_pool.tile([C, CW], F32)
                r0 = r0_pool.tile([1, PW], F32)
                r1 = r1_pool.tile([1, PW], F32)
                sp_live = {}
                emit_s(c, 0, sp_live)
                for m in range(MT):
                    # keep PE one iteration ahead of the exp on ScalarE
                    if m + 1 < MT:
                        emit_s(c, m + 1, sp_live)
                    pt = pt_pool.tile([C, CW], BF16)
                    nc.scalar.activation(pt, sp_live.pop(m), AF.Exp)
                    st = m == 0
                    en = m == MT - 1
                    nc.tensor.matmul(o_ps[:, 0:PW], vt_sb[:, m * C:(m + 1) * C], pt[:, 0:PW],
                                     start=st, stop=en)
                    nc.tensor.matmul(o_ps[:, PW:CW], vt_sb[:, m * C:(m + 1) * C], pt[:, PW:CW],
                                     start=st, stop=en)
                    nc.tensor.matmul(r0, ones1, pt[:, 0:PW], start=st, stop=en)
                    nc.tensor.matmul(r1, ones1, pt[:, PW:CW], start=st, stop=en)
                    if m == 4 and pending is not None:
                        # emit the previous chunk's tail here so it executes
                        # overlapped with this chunk's m-loop
                        emit_tail(*pending)
                        pending = None
                pending = (c, o_ps, r0, r1)
            emit_tail(*pending)


def build():
    nc = bacc.Bacc("TRN2", target_bir_lowering=False, debug=False, num_devices=B)
    x_d = nc.dram_tensor("x", [C, HW], F32, kind="ExternalInput").ap()
    gamma_d = nc.dram_tensor("gamma", [C, 1], F32, kind="ExternalInput").ap()
    beta_d = nc.dram_tensor("beta", [C, 1], F32, kind="ExternalInput").ap()
    wqkvT_d = nc.dram_tensor("wqkvT", [C, 3 * C], F32, kind="ExternalInput").ap()
    bqkv3_d = nc.dram_tensor("bqkv3", [C, 3], F32, kind="ExternalInput").ap()
    woutT_d = nc.dram_tensor("woutT", [C, C], F32, kind="ExternalInput").ap()
    bout_d = nc.dram_tensor("bout", [C, 1], F32, kind="ExternalInput").ap()
    out_d = nc.dram_tensor("out", [C, HW], F32, kind="ExternalOutput").ap()

    G = np.zeros((C, NG), np.float32)
    G[np.arange(C), np.arange(C) // GS] = 1.0 / GS
    GT = np.zeros((NG, C), np.float32)
    GT[np.arange(C) // GS, np.arange(C)] = 1.0
    G_d = nc.inline_tensor(G, "Gmat").ap()
    GT_d = nc.inline_tensor(GT, "GTmat").ap()

    with tile.TileContext(nc) as tc:
        _body(tc, x_d, gamma_d, beta_d, wqkvT_d, bqkv3_d, woutT_d, bout_d, G_d, GT_d, out_d)
    nc.compile()
    return nc


_NC = None


def _get_nc():
    global _NC
    if _NC is None:
        _NC = build()
    return _NC


def _in_maps(x, gamma, beta, w_qkv, b_qkv, w_out, b_out):
    x = np.ascontiguousarray(np.asarray(x, np.float32)).reshape(B, C, HW)
    shared = {
        "gamma": np.ascontiguousarray(np.asarray(gamma, np.float32).reshape(C, 1)),
        "beta": np.ascontiguousarray(np.asarray(beta, np.float32).reshape(C, 1)),
        "wqkvT": np.ascontiguousarray(np.asarray(w_qkv, np.float32).T),
        "bqkv3": np.ascontiguousarray(np.asarray(b_qkv, np.float32).reshape(3, C).T),
        "woutT": np.ascontiguousarray(np.asarray(w_out, np.float32).T),
        "bout": np.ascontiguousarray(np.asarray(b_out, np.float32).reshape(C, 1)),
    }
    return [{"x": np.ascontiguousarray(x[b]), **shared} for b in range(B)]


def run(inputs, trace=False, **trace_kwargs):
    nc = _get_nc()
    maps = _in_maps(**inputs)
    res = run_bass_kernel_spmd(nc, maps, list(range(B)), trace=trace, **trace_kwargs)
    out = np.stack([np.asarray(res.results[i]["out"]) for i in range(B)])
    return out.reshape(B, C, H, W).astype(np.float32), res


def kernel(**inputs):
    out, _ = run(inputs)
    return out
